# revision 2
# baseline (speedup 1.0000x reference)
"""Bass/Trainium2 kernel for nn_HMEClassification (hierarchical mixture-of-experts).

Strategy: pure data parallel across 8 cores (batch sharded). Per core:
  xT [128d, 16384b] streamed in 512-wide b-tiles (bf16).
  L1 (7 units: 3 gates + 4 experts): weight-stationary bf16 matmuls
      lhsT=W1 block [128d,128h], rhs=xT tile [128d,512b] -> PSUM [128h,512b].
      Evacuated PSUM->SBUF bf16 with fused relu over [128,1024] 2-bank pairs
      (L1 biases are zero per spec), split Scalar/Vector.
  Gates: all three gate logit-diff matmuls land in ONE psum bank (psGa) on
      DISJOINT PE column tiles so they stream concurrently:
        G1 (root, +-dR dup) rows {0,1,32,33} @ tile (0,0)  [lhsT 128x34]
        GA (+dA,-dA)        rows {64,65}     @ tile (0,64) [lhsT 128x2]
        GB (+dB,-dB)        rows {96,97}     @ tile (0,96) [lhsT 128x2]
      One fused Scalar op E = exp(-psGa[0:98]-db_pat) evacuates all of them.
  L2 experts: col-tiled pairs (0,0)/(0,64) k-accumulated into ONE [128,1024]
      psum tile (pair0 cols 0:512, pair1 512:1024); single Scalar exp evac.
  Softmax sums: two ones-select matmuls on concurrent tiles (0,0)/(0,32)
      into psGb rows {0,1} / {32,33}.
  Combine C = 1/((1+E1)(1+E2)S) rows {0,1,32,33} (two stt + recip on DVE,
      bf16 copy on DVE 2x path).
  Partition-broadcast of C via block-ones matmuls re-using the psE banks;
      prod = expc * bcast(C) as ONE [128,1024] DVE op; final 4-expert sum via
      stacked-identity matmuls into psGb rows 64-127; Scalar evac; DMA out.
  Output out^T [64, 16384] fp32 per core; host transposes/concats.
"""

import ml_dtypes
import numpy as np

import concourse.bass as bass
import concourse.mybir as mybir
import concourse.tile as tile
from concourse import bacc
from concourse.bass_utils import run_bass_kernel_spmd

B, D, H, C = 131072, 128, 512, 64
NCORES = 8
BC = B // NCORES        # 16384 rows per core
TB = 512                # b-tile width
KH = H // 128           # 4 h-chunks of 128

F32 = mybir.dt.float32
BF16 = mybir.dt.bfloat16

# ---- bf16 consts layout (columns in [128, NB] bf16 tensor) ----
W1_OFF = 0                       # 7 units * 512 = 3584
W2_OFF = W1_OFF + 7 * H          # 16 blocks (k*4+e) * 64 = 1024
GR_OFF = W2_OFF + 16 * 64        # 4 chunks * 34 (root merged +/-)
GA_OFF = GR_OFF + 4 * 34         # 4 chunks * 2 (A: +v,-v)
GB_OFF2 = GA_OFF + 4 * 2         # 4 chunks * 2 (B: +v,-v)
OS_OFF = GB_OFF2 + 4 * 2         # 2 cols (ones select)
BC_OFF = OS_OFF + 2              # 128 cols (partition-broadcast lhsT)
ID_OFF = BC_OFF + 128            # 64 cols (stacked identity)
NB = ID_OFF + 64
# ---- fp32 consts layout ----
GE_OFF = 0                       # 1 col: -bias pattern for gate exp (98 rows)
NF = GE_OFF + 1


def _build_consts(gW1, gb1, gW2, gb2, eW1, eb1, eW2, eb2):
    cb = np.zeros((128, NB), dtype=np.float32)
    for u in range(3):
        cb[:, W1_OFF + u * H: W1_OFF + (u + 1) * H] = gW1[u]
    for e in range(4):
        cb[:, W1_OFF + (3 + e) * H: W1_OFF + (4 + e) * H] = eW1[e]
    for k in range(KH):
        for e in range(4):
            cb[:, W2_OFF + (k * 4 + e) * 64: W2_OFF + (k * 4 + e + 1) * 64] = \
                eW2[e, k * 128:(k + 1) * 128, :]
    v = gW2[:, :, 0] - gW2[:, :, 1]          # [3, 512] logit-diff weights
    for k in range(KH):
        sl = slice(k * 128, (k + 1) * 128)
        blk = np.zeros((128, 34), dtype=np.float32)
        blk[:, 0] = v[0, sl]
        blk[:, 1] = v[0, sl]
        blk[:, 32] = -v[0, sl]
        blk[:, 33] = -v[0, sl]
        cb[:, GR_OFF + k * 34: GR_OFF + (k + 1) * 34] = blk
        cb[:, GA_OFF + k * 2] = v[1, sl]
        cb[:, GA_OFF + k * 2 + 1] = -v[1, sl]
        cb[:, GB_OFF2 + k * 2] = v[2, sl]
        cb[:, GB_OFF2 + k * 2 + 1] = -v[2, sl]
    cb[:64, OS_OFF + 0] = 1.0
    cb[64:, OS_OFF + 1] = 1.0
    # broadcast lhsT [2,128]: row0 -> out partitions 0-63, row1 -> 64-127.
    # Replicated at rows 32,33 (matmul needs lhsT/rhs base partitions equal).
    for r0 in (0, 32):
        cb[r0, BC_OFF: BC_OFF + 64] = 1.0
        cb[r0 + 1, BC_OFF + 64: BC_OFF + 128] = 1.0
    p = np.arange(128)
    cb[:, ID_OFF: ID_OFF + 64] = (p[:, None] % 64 == np.arange(64)[None, :])

    # gate exp bias pattern (gb2 diffs; zeros per spec but kept for exactness)
    cf = np.zeros((128, NF), dtype=np.float32)
    db = gb2[:, 0] - gb2[:, 1]               # [3]
    cf[0:2, GE_OFF] = -db[0]
    cf[32:34, GE_OFF] = db[0]
    cf[64, GE_OFF] = -db[1]
    cf[65, GE_OFF] = db[1]
    cf[96, GE_OFF] = -db[2]
    cf[97, GE_OFF] = db[2]
    return cb.astype(ml_dtypes.bfloat16), cf


def _build_nc(n_tiles):
    nc = bacc.Bacc("TRN2", target_bir_lowering=False)
    xt = nc.dram_tensor("xt", [D, BC], BF16, kind="ExternalInput")
    cbd = nc.dram_tensor("cb", [128, NB], BF16, kind="ExternalInput")
    cfd = nc.dram_tensor("cf", [128, NF], F32, kind="ExternalInput")
    outT = nc.dram_tensor("outT", [C, BC], F32, kind="ExternalOutput")

    AF = mybir.ActivationFunctionType
    OP = mybir.AluOpType

    with tile.TileContext(nc) as tc:
        with (
            tc.tile_pool(name="singles", bufs=1) as singles,
            tc.tile_pool(name="xp", bufs=3) as xp,
            tc.tile_pool(name="hp", bufs=3) as hp,
            tc.tile_pool(name="ep", bufs=2) as ep,
            tc.tile_pool(name="sp", bufs=3) as sp,
            tc.tile_pool(name="op", bufs=2) as op_pool,
            tc.tile_pool(name="psL1", bufs=2, space="PSUM") as psL1p,
            tc.tile_pool(name="psE", bufs=1, space="PSUM") as psEp,
            tc.tile_pool(name="psG", bufs=1, space="PSUM") as psGp,
        ):
            cs = singles.tile([128, NB], BF16)
            nc.sync.dma_start(out=cs, in_=cbd[:, :])
            cf = singles.tile([128, NF], F32)
            nc.sync.dma_start(out=cf, in_=cfd[:, :])

            def w1_ap(u, hb):
                a = W1_OFF + u * H + hb * 128
                return cs[:, a: a + 128]

            def w2_ap(k, e):
                a = W2_OFF + (k * 4 + e) * 64
                return cs[:, a: a + 64]

            for t in range(n_tiles):
                xtile = xp.tile([D, TB], BF16, tag="x")
                nc.sync.dma_start(out=xtile, in_=xt[:, t * TB:(t + 1) * TB])

                # ---- L1: 7 units x 4 h-blocks, 2-bank double tiles ----
                # Gate units (j 0..5) first so gate matmuls + E evac start
                # early; expert units (j 6..13) after.  L1 biases are zero
                # (spec fill=zeros) so each [128,1024] pair evacuates in ONE
                # relu op, split across Scalar (8) and Vector (6).
                hsb = {}

                def l1_pair(j):
                    u, hb0 = (2 * j) // KH, (2 * j) % KH
                    psD = psL1p.tile([128, 2 * TB], F32, tag="l1")
                    nc.tensor.matmul(psD[:, 0:TB], w1_ap(u, hb0), xtile,
                                     start=True, stop=True)
                    nc.tensor.matmul(psD[:, TB:2 * TB], w1_ap(u, hb0 + 1),
                                     xtile, start=True, stop=True)
                    hd = hp.tile([128, 2 * TB], BF16, tag=f"h{j}", bufs=3)
                    if j in (2, 4, 6, 8, 10, 12):
                        nc.vector.tensor_scalar(hd, psD, 0.0, None, op0=OP.max)
                    else:
                        nc.scalar.activation(hd, psD, AF.Relu)
                    hsb[u, hb0] = hd[:, 0:TB]
                    hsb[u, hb0 + 1] = hd[:, TB:2 * TB]

                for j in range(6):
                    l1_pair(j)

                # ---- gates: one psum bank, three concurrent PE col tiles ----
                # psGa rows {0,1}=+dR,+dR {32,33}=-dR,-dR  (tile (0,0), M=34)
                #      rows {64,65}=+dA,-dA                (tile (0,64))
                #      rows {96,97}=+dB,-dB                (tile (0,96))
                psGa = psGp.tile([128, TB], F32, tag="ga")
                for k in range(KH):
                    st, sp_ = (k == 0), (k == KH - 1)
                    nc.tensor.matmul(psGa[0:34, :],
                                     cs[:, GR_OFF + k * 34: GR_OFF + (k + 1) * 34],
                                     hsb[0, k], start=st, stop=sp_,
                                     tile_position=(0, 0))
                    nc.tensor.matmul(psGa[64:66, :],
                                     cs[:, GA_OFF + k * 2: GA_OFF + (k + 1) * 2],
                                     hsb[1, k], start=st, stop=sp_,
                                     tile_position=(0, 64))
                    nc.tensor.matmul(psGa[96:98, :],
                                     cs[:, GB_OFF2 + k * 2: GB_OFF2 + (k + 1) * 2],
                                     hsb[2, k], start=st, stop=sp_,
                                     tile_position=(0, 96))

                # one fused exp over all gate rows (rows 34-63 are unused)
                E = sp.tile([98, TB], F32, tag="E")
                nc.scalar.activation(E, psGa[0:98, :], AF.Exp, scale=-1.0,
                                     bias=cf[0:98, GE_OFF: GE_OFF + 1])

                for j in range(6, 14):
                    l1_pair(j)

                # ---- L2 experts: pairs (e0,e1) and (e2,e3) -> ONE [128,1024]
                # psum tile; pair p in cols p*TB:(p+1)*TB ----
                psE2 = psEp.tile([128, 2 * TB], F32, tag="z")
                for pair in range(2):
                    ua, ub = 3 + 2 * pair, 4 + 2 * pair
                    dst = psE2[:, pair * TB:(pair + 1) * TB]
                    for k in range(KH):
                        nc.tensor.matmul(dst[0:64, :], w2_ap(k, 2 * pair),
                                         hsb[ua, k], start=(k == 0),
                                         stop=(k == KH - 1),
                                         tile_position=(0, 0))
                        nc.tensor.matmul(dst[64:128, :], w2_ap(k, 2 * pair + 1),
                                         hsb[ub, k], start=(k == 0),
                                         stop=(k == KH - 1),
                                         tile_position=(0, 64))
                # single exp evac for both pairs (eb2 is zero per spec)
                expc = ep.tile([128, 2 * TB], BF16, tag="exp")
                nc.scalar.activation(expc, psE2, AF.Exp)

                # ---- softmax sums on concurrent tiles (0,0)/(0,32) ----
                # psGb rows {0,1}=S_A1,S_A2  rows {32,33}=S_B1,S_B2
                psGb = psGp.tile([128, TB], F32, tag="gb")
                nc.tensor.matmul(psGb[0:2, :], cs[:, OS_OFF: OS_OFF + 2],
                                 expc[:, 0:TB], start=True, stop=True,
                                 tile_position=(0, 0))
                nc.tensor.matmul(psGb[32:34, :], cs[:, OS_OFF: OS_OFF + 2],
                                 expc[:, TB:2 * TB], start=True, stop=True,
                                 tile_position=(0, 32))

                # ---- combine coeffs C = 1/((1+E1)(1+E2)S), rows {0,1,32,33} ----
                t34 = sp.tile([34, TB], F32, tag="t34")
                nc.vector.scalar_tensor_tensor(t34, E[64:98, :], 1.0,
                                               psGb[0:34, :],
                                               op0=OP.add, op1=OP.mult)
                m34 = sp.tile([34, TB], F32, tag="m34")
                nc.vector.scalar_tensor_tensor(m34, E[0:34, :], 1.0, t34,
                                               op0=OP.add, op1=OP.mult)
                Cf_t = sp.tile([34, TB], F32, tag="C")
                nc.vector.reciprocal_approx_fast(Cf_t, m34)
                Cb = sp.tile([34, TB], BF16, tag="Cb")
                nc.vector.tensor_scalar(Cb, Cf_t, 0.0, None, op0=OP.add)

                # ---- partition-broadcast of coeff rows via PE matmul,
                # re-using the psE2 banks (expc already evacuated) ----
                psBC = psEp.tile([128, 2 * TB], F32, tag="z")
                for pair in range(2):
                    bl = cs[32 * pair: 32 * pair + 2, BC_OFF: BC_OFF + 128]
                    nc.tensor.matmul(psBC[:, pair * TB:(pair + 1) * TB], bl,
                                     Cb[32 * pair: 32 * pair + 2, :],
                                     start=True, stop=True)
                # one [128,1024] product op
                prod = sp.tile([128, 2 * TB], BF16, tag="prod")
                nc.vector.tensor_tensor(prod, expc, psBC, op=OP.mult)

                # ---- final sum of 4 experts via stacked identity into
                # psGb rows 64-127 (tile (0,64)) ----
                psO = psGb[64:128, :]
                id2 = cs[:, ID_OFF: ID_OFF + 64]
                nc.tensor.matmul(psO, id2, prod[:, 0:TB], start=True,
                                 stop=False, tile_position=(0, 64))
                nc.tensor.matmul(psO, id2, prod[:, TB:2 * TB], start=False,
                                 stop=True, tile_position=(0, 64))
                osb = op_pool.tile([64, TB], F32, tag="osb")
                nc.scalar.copy(osb, psO)
                nc.sync.dma_start(out=outT[:, t * TB:(t + 1) * TB], in_=osb)

    nc.compile()
    return nc


def kernel(x, gW1, gb1, gW2, gb2, eW1, eb1, eW2, eb2, _trace=False):
    x = np.asarray(x, dtype=np.float32)
    cb, cf = _build_consts(
        np.asarray(gW1, np.float32), np.asarray(gb1, np.float32),
        np.asarray(gW2, np.float32), np.asarray(gb2, np.float32),
        np.asarray(eW1, np.float32), np.asarray(eb1, np.float32),
        np.asarray(eW2, np.float32), np.asarray(eb2, np.float32))
    n_rows = x.shape[0]
    bc = n_rows // NCORES
    n_tiles = bc // TB
    assert bc * NCORES == n_rows and n_tiles * TB == bc

    global BC
    BC = bc
    nc = _build_nc(n_tiles)

    xs = x.reshape(NCORES, bc, D)
    in_maps = [
        {"xt": np.ascontiguousarray(xs[c].T).astype(ml_dtypes.bfloat16),
         "cb": cb, "cf": cf}
        for c in range(NCORES)
    ]
    res = run_bass_kernel_spmd(nc, in_maps, core_ids=list(range(NCORES)),
                               trace=_trace)
    out = np.concatenate([r["outT"].T for r in res.results], axis=0)
    kernel.last_results = res
    return np.ascontiguousarray(out.astype(np.float32))


# revision 3
# speedup vs baseline: 1.3658x; 1.3658x over previous
"""Bass/Trainium2 kernel for nn_HMEClassification (hierarchical mixture-of-experts).

Strategy: pure data parallel across 8 cores (batch sharded). Per core:
  xT [128d, 16384b] streamed in 512-wide b-tiles (bf16).
  L1 (7 units: 3 gates + 4 experts): weight-stationary bf16 matmuls
      lhsT=W1 block [128d,128h], rhs=xT tile [128d,512b] -> PSUM [128h,512b].
      Evacuated PSUM->SBUF bf16 with fused relu over [128,1024] 2-bank pairs
      (L1 biases are zero per spec), split 8 Scalar / 6 Vector.
  Gates: all three gate logit-diff matmuls land in ONE psum bank (psGa) on
      DISJOINT PE column tiles so they stream concurrently (col tiling):
        G1 (root, +-dR dup) rows {0,1,32,33} @ tile (0,0)  [lhsT 128x34]
        GA (+dA,-dA)        rows {64,65}     @ tile (0,64) [lhsT 128x2]
        GB (+dB,-dB)        rows {96,97}     @ tile (0,96) [lhsT 128x2]
      One fused Scalar op E = exp(-psGa[0:98]-db_pat) evacuates all six gate
      exponentials at once.
  L2 experts: col-tiled pairs (0,0)/(0,64), K-accumulated over 4 h-chunks
      -> PSUM [128(=2x64c), 512b]; evac with Exp activation (eb2 zero).
  Softmax sums: two ones-select matmuls on concurrent tiles (0,0)/(0,32)
      into psGb rows {0,1} / {32,33}.
  Combine C = 1/((1+E1)(1+E2)S) rows {0,1,32,33}: two fused stt + one
      reciprocal_approx_fast + bf16 convert, all on Vector.
  Partition-broadcast of C rows via block-ones PE matmuls into the psE bank
      rotation; prod = exp * C (bf16); final 4-expert sum via stacked
      identity into psGb rows 64-127; Scalar evac; DMA out.
  Output out^T [64, 16384] fp32 per core; host transposes/concats.
"""

import ml_dtypes
import numpy as np

import concourse.bass as bass
import concourse.mybir as mybir
import concourse.tile as tile
from concourse import bacc
from concourse.bass_utils import run_bass_kernel_spmd

B, D, H, C = 131072, 128, 512, 64
NCORES = 8
BC = B // NCORES        # 16384 rows per core
TB = 512                # b-tile width
KH = H // 128           # 4 h-chunks of 128

F32 = mybir.dt.float32
BF16 = mybir.dt.bfloat16

# ---- bf16 consts layout (columns in [128, NB] bf16 tensor) ----
W1_OFF = 0                       # 7 units * 512 = 3584
W2_OFF = W1_OFF + 7 * H          # 16 blocks (k*4+e) * 64 = 1024
GR_OFF = W2_OFF + 16 * 64        # 4 chunks * 34 (root merged +/-)
GA_OFF = GR_OFF + 4 * 34         # 4 chunks * 2 (A: +v,-v)
GB_OFF2 = GA_OFF + 4 * 2         # 4 chunks * 2 (B: +v,-v)
OS_OFF = GB_OFF2 + 4 * 2         # 2 cols (ones select)
BC_OFF = OS_OFF + 2              # 128 cols (partition-broadcast lhsT)
ID_OFF = BC_OFF + 128            # 64 cols (stacked identity)
NB = ID_OFF + 64
# ---- fp32 consts layout ----
GE_OFF = 0                       # 1 col: -bias pattern for gate exp (98 rows)
NF = GE_OFF + 1


def _build_consts(gW1, gb1, gW2, gb2, eW1, eb1, eW2, eb2):
    cb = np.zeros((128, NB), dtype=np.float32)
    for u in range(3):
        cb[:, W1_OFF + u * H: W1_OFF + (u + 1) * H] = gW1[u]
    for e in range(4):
        cb[:, W1_OFF + (3 + e) * H: W1_OFF + (4 + e) * H] = eW1[e]
    for k in range(KH):
        for e in range(4):
            cb[:, W2_OFF + (k * 4 + e) * 64: W2_OFF + (k * 4 + e + 1) * 64] = \
                eW2[e, k * 128:(k + 1) * 128, :]
    v = gW2[:, :, 0] - gW2[:, :, 1]          # [3, 512] logit-diff weights
    for k in range(KH):
        sl = slice(k * 128, (k + 1) * 128)
        blk = np.zeros((128, 34), dtype=np.float32)
        blk[:, 0] = v[0, sl]
        blk[:, 1] = v[0, sl]
        blk[:, 32] = -v[0, sl]
        blk[:, 33] = -v[0, sl]
        cb[:, GR_OFF + k * 34: GR_OFF + (k + 1) * 34] = blk
        cb[:, GA_OFF + k * 2] = v[1, sl]
        cb[:, GA_OFF + k * 2 + 1] = -v[1, sl]
        cb[:, GB_OFF2 + k * 2] = v[2, sl]
        cb[:, GB_OFF2 + k * 2 + 1] = -v[2, sl]
    cb[:64, OS_OFF + 0] = 1.0
    cb[64:, OS_OFF + 1] = 1.0
    # broadcast lhsT [2,128]: row0 -> out partitions 0-63, row1 -> 64-127.
    # Replicated at rows 32,33 (matmul needs lhsT/rhs base partitions equal).
    for r0 in (0, 32):
        cb[r0, BC_OFF: BC_OFF + 64] = 1.0
        cb[r0 + 1, BC_OFF + 64: BC_OFF + 128] = 1.0
    p = np.arange(128)
    cb[:, ID_OFF: ID_OFF + 64] = (p[:, None] % 64 == np.arange(64)[None, :])

    # gate exp bias pattern (gb2 diffs; zeros per spec but kept for exactness)
    cf = np.zeros((128, NF), dtype=np.float32)
    db = gb2[:, 0] - gb2[:, 1]               # [3]
    cf[0:2, GE_OFF] = -db[0]
    cf[32:34, GE_OFF] = db[0]
    cf[64, GE_OFF] = -db[1]
    cf[65, GE_OFF] = db[1]
    cf[96, GE_OFF] = -db[2]
    cf[97, GE_OFF] = db[2]
    return cb.astype(ml_dtypes.bfloat16), cf


def _build_nc(n_tiles):
    nc = bacc.Bacc("TRN2", target_bir_lowering=False)
    xt = nc.dram_tensor("xt", [D, BC], BF16, kind="ExternalInput")
    cbd = nc.dram_tensor("cb", [128, NB], BF16, kind="ExternalInput")
    cfd = nc.dram_tensor("cf", [128, NF], F32, kind="ExternalInput")
    outT = nc.dram_tensor("outT", [C, BC], F32, kind="ExternalOutput")

    AF = mybir.ActivationFunctionType
    OP = mybir.AluOpType

    with tile.TileContext(nc) as tc:
        with (
            tc.tile_pool(name="singles", bufs=1) as singles,
            tc.tile_pool(name="xp", bufs=3) as xp,
            tc.tile_pool(name="hp", bufs=3) as hp,
            tc.tile_pool(name="ep", bufs=2) as ep,
            tc.tile_pool(name="sp", bufs=3) as sp,
            tc.tile_pool(name="op", bufs=2) as op_pool,
            tc.tile_pool(name="psL1", bufs=2, space="PSUM") as psL1p,
            tc.tile_pool(name="psE", bufs=2, space="PSUM") as psEp,
            tc.tile_pool(name="psG", bufs=1, space="PSUM") as psGp,
        ):
            cs = singles.tile([128, NB], BF16)
            nc.sync.dma_start(out=cs, in_=cbd[:, :])
            cf = singles.tile([128, NF], F32)
            nc.sync.dma_start(out=cf, in_=cfd[:, :])

            def w1_ap(u, hb):
                a = W1_OFF + u * H + hb * 128
                return cs[:, a: a + 128]

            def w2_ap(k, e):
                a = W2_OFF + (k * 4 + e) * 64
                return cs[:, a: a + 64]

            for t in range(n_tiles):
                xtile = xp.tile([D, TB], BF16, tag="x")
                nc.sync.dma_start(out=xtile, in_=xt[:, t * TB:(t + 1) * TB])

                # ---- L1: 7 units x 4 h-blocks, 2-bank double tiles ----
                # L1 biases are zero (spec fill=zeros), so each [128,1024]
                # PSUM pair evacuates in ONE relu op (8 Scalar / 6 Vector).
                hsb = {}
                for j in range(14):
                    u, hb0 = (2 * j) // KH, (2 * j) % KH
                    psD = psL1p.tile([128, 2 * TB], F32, tag="l1")
                    nc.tensor.matmul(psD[:, 0:TB], w1_ap(u, hb0), xtile,
                                     start=True, stop=True)
                    nc.tensor.matmul(psD[:, TB:2 * TB], w1_ap(u, hb0 + 1),
                                     xtile, start=True, stop=True)
                    hd = hp.tile([128, 2 * TB], BF16, tag=f"h{j}", bufs=3)
                    if j in (2, 4, 6, 8, 10, 12):
                        nc.vector.tensor_scalar(hd, psD, 0.0, None, op0=OP.max)
                    else:
                        nc.scalar.activation(hd, psD, AF.Relu)
                    hsb[u, hb0] = hd[:, 0:TB]
                    hsb[u, hb0 + 1] = hd[:, TB:2 * TB]

                # ---- gates: one psum bank, three concurrent PE col tiles ----
                psGa = psGp.tile([128, TB], F32, tag="ga")
                for k in range(KH):
                    st, sp_ = (k == 0), (k == KH - 1)
                    nc.tensor.matmul(psGa[0:34, :],
                                     cs[:, GR_OFF + k * 34: GR_OFF + (k + 1) * 34],
                                     hsb[0, k], start=st, stop=sp_,
                                     tile_position=(0, 0))
                    nc.tensor.matmul(psGa[64:66, :],
                                     cs[:, GA_OFF + k * 2: GA_OFF + (k + 1) * 2],
                                     hsb[1, k], start=st, stop=sp_,
                                     tile_position=(0, 64))
                    nc.tensor.matmul(psGa[96:98, :],
                                     cs[:, GB_OFF2 + k * 2: GB_OFF2 + (k + 1) * 2],
                                     hsb[2, k], start=st, stop=sp_,
                                     tile_position=(0, 96))

                # one fused exp over all gate rows (rows 34-63 are unused)
                E = sp.tile([98, TB], F32, tag="E")
                nc.scalar.activation(E, psGa[0:98, :], AF.Exp, scale=-1.0,
                                     bias=cf[0:98, GE_OFF: GE_OFF + 1])

                # ---- L2 experts: pairs (e0,e1) and (e2,e3), col-tiled ----
                expc = ep.tile([128, 2 * TB], BF16, tag="exp")
                for pair in range(2):
                    psE = psEp.tile([128, TB], F32, tag="e2")
                    ua, ub = 3 + 2 * pair, 4 + 2 * pair
                    for k in range(KH):
                        nc.tensor.matmul(psE[0:64, :], w2_ap(k, 2 * pair),
                                         hsb[ua, k], start=(k == 0),
                                         stop=(k == KH - 1),
                                         tile_position=(0, 0))
                        nc.tensor.matmul(psE[64:128, :], w2_ap(k, 2 * pair + 1),
                                         hsb[ub, k], start=(k == 0),
                                         stop=(k == KH - 1),
                                         tile_position=(0, 64))
                    # eb2 is zero per spec -> plain exp evac
                    nc.scalar.activation(expc[:, pair * TB:(pair + 1) * TB],
                                         psE, AF.Exp)

                # ---- softmax sums on concurrent tiles (0,0)/(0,32) ----
                # psGb rows {0,1}=S_A1,S_A2  rows {32,33}=S_B1,S_B2
                psGb = psGp.tile([128, TB], F32, tag="gb")
                nc.tensor.matmul(psGb[0:2, :], cs[:, OS_OFF: OS_OFF + 2],
                                 expc[:, 0:TB], start=True, stop=True,
                                 tile_position=(0, 0))
                nc.tensor.matmul(psGb[32:34, :], cs[:, OS_OFF: OS_OFF + 2],
                                 expc[:, TB:2 * TB], start=True, stop=True,
                                 tile_position=(0, 32))

                # ---- combine coeffs C = 1/((1+E1)(1+E2)S), rows {0,1,32,33} ----
                t34 = sp.tile([34, TB], F32, tag="t34")
                nc.vector.scalar_tensor_tensor(t34, E[64:98, :], 1.0,
                                               psGb[0:34, :],
                                               op0=OP.add, op1=OP.mult)
                m34 = sp.tile([34, TB], F32, tag="m34")
                nc.vector.scalar_tensor_tensor(m34, E[0:34, :], 1.0, t34,
                                               op0=OP.add, op1=OP.mult)
                Cf_t = sp.tile([34, TB], F32, tag="C")
                nc.vector.reciprocal_approx_fast(Cf_t, m34)
                Cb = sp.tile([34, TB], BF16, tag="Cb")
                nc.vector.tensor_scalar(Cb, Cf_t, 0.0, None, op0=OP.add)

                # ---- partition-broadcast of coeff rows via PE matmul ----
                # psBC reuses the psE rotation (exp already evacuated).
                prods = []
                for pair in range(2):
                    psBC = psEp.tile([128, TB], F32, tag="e2")
                    bl = cs[32 * pair: 32 * pair + 2, BC_OFF: BC_OFF + 128]
                    nc.tensor.matmul(psBC, bl, Cb[32 * pair: 32 * pair + 2, :],
                                     start=True, stop=True)
                    prod = sp.tile([128, TB], BF16, tag=f"prod{pair}")
                    nc.vector.tensor_tensor(
                        prod, expc[:, pair * TB:(pair + 1) * TB], psBC,
                        op=OP.mult)
                    prods.append(prod)

                # ---- final sum of 4 experts via stacked identity into
                # psGb rows 64-127 (tile (0,64)) ----
                psO = psGb[64:128, :]
                id2 = cs[:, ID_OFF: ID_OFF + 64]
                nc.tensor.matmul(psO, id2, prods[0], start=True, stop=False,
                                 tile_position=(0, 64))
                nc.tensor.matmul(psO, id2, prods[1], start=False, stop=True,
                                 tile_position=(0, 64))
                osb = op_pool.tile([64, TB], F32, tag="osb")
                nc.scalar.copy(osb, psO)
                nc.sync.dma_start(out=outT[:, t * TB:(t + 1) * TB], in_=osb)

    nc.compile()
    return nc


def kernel(x, gW1, gb1, gW2, gb2, eW1, eb1, eW2, eb2, _trace=False):
    x = np.asarray(x, dtype=np.float32)
    cb, cf = _build_consts(
        np.asarray(gW1, np.float32), np.asarray(gb1, np.float32),
        np.asarray(gW2, np.float32), np.asarray(gb2, np.float32),
        np.asarray(eW1, np.float32), np.asarray(eb1, np.float32),
        np.asarray(eW2, np.float32), np.asarray(eb2, np.float32))
    n_rows = x.shape[0]
    bc = n_rows // NCORES
    n_tiles = bc // TB
    assert bc * NCORES == n_rows and n_tiles * TB == bc

    global BC
    BC = bc
    nc = _build_nc(n_tiles)

    xs = x.reshape(NCORES, bc, D)
    in_maps = [
        {"xt": np.ascontiguousarray(xs[c].T).astype(ml_dtypes.bfloat16),
         "cb": cb, "cf": cf}
        for c in range(NCORES)
    ]
    res = run_bass_kernel_spmd(nc, in_maps, core_ids=list(range(NCORES)),
                               trace=_trace)
    out = np.concatenate([r["outT"].T for r in res.results], axis=0)
    kernel.last_results = res
    return np.ascontiguousarray(out.astype(np.float32))


# revision 7
# speedup vs baseline: 1.4687x; 1.0754x over previous
"""Bass/Trainium2 kernel for nn_HMEClassification (hierarchical mixture-of-experts).

Strategy: pure data parallel across 8 cores (batch sharded). Per core:
  xT [128d, 16384b] streamed in 512-wide b-tiles (bf16).

  The PE array drains whenever the tile MODE (row/col tiling config) changes,
  so the loop body is software-pipelined BY HAND with a 2-iteration skew so
  that same-mode matmuls are adjacent and stream concurrently on disjoint
  column tiles:

    block t:
      1. L1(t): 28x (128,128)-mode matmuls (7 units x 4 h-chunks), evac'd
         as [128,1024] relu pairs (8 Scalar / 6 Vector; L1 biases zero).
      2. (128,32)-mode group: gates(t) 16 matmuls on four concurrent col
         tiles (G1a/G1b/GA/GB at (0,0)/(0,32)/(0,64)/(0,96), all M=2,
         4 k-chunks each, all into ONE psum bank psGa rows
         {0,1},{32,33},{64,65},{96,97}) + softmax sums(t-1) on (0,0)/(0,32)
         into psGb rows {0,1},{32,33}.
      3. E(t) = exp(-psGa[0:98]) one fused Scalar op; coeff chain (t-1) on
         Vector: t34=(E2+1)S, m34=(E1+1)t34, C=1/m34, Cb=bf16(C).
      4. (128,64)-mode group: experts(t) 16 matmuls (pairs on (0,0)/(0,64),
         K-accumulated) + final(t-2) stacked-identity sum into psGb rows
         64-127; osb(t-2) Scalar evac; DMA out.
      5. expc(t) = exp(psE) Scalar evacs.
      6. (32,128)-row-mode: bcast(t-1) C rows via block-ones matmuls into
         the psE rotation (rows {0,1}/{32,33} -> concurrent row tiles);
         prod(t-1) = expc*bcast on Vector.

  C = 1/((1+E1)(1+E2)S) packs all four gate combos in rows {0,1,32,33}.
  Output out^T [64, 16384] fp32 per core; host transposes/concats.
"""

import ml_dtypes
import numpy as np

import concourse.bass as bass
import concourse.mybir as mybir
import concourse.tile as tile
from concourse import bacc
from concourse.bass_utils import run_bass_kernel_spmd

B, D, H, C = 131072, 128, 512, 64
NCORES = 8
BC = B // NCORES        # 16384 rows per core
TB = 512                # b-tile width
KH = H // 128           # 4 h-chunks of 128

F32 = mybir.dt.float32
BF16 = mybir.dt.bfloat16

# ---- bf16 consts layout (columns in [128, NB] bf16 tensor) ----
W1_OFF = 0                       # 7 units * 512 = 3584
W2_OFF = W1_OFF + 7 * H          # 16 blocks (k*4+e) * 64 = 1024
GR_OFF = W2_OFF + 16 * 64        # 4 chunks * 34 (root +/- at cols {0,1},{32,33})
GA_OFF = GR_OFF + 4 * 34         # 4 chunks * 2 (A: +v,-v)
GB_OFF2 = GA_OFF + 4 * 2         # 4 chunks * 2 (B: +v,-v)
OS_OFF = GB_OFF2 + 4 * 2         # 2 cols (ones select)
BC_OFF = OS_OFF + 2              # 128 cols (partition-broadcast lhsT)
ID_OFF = BC_OFF + 128            # 64 cols (stacked identity)
NB = ID_OFF + 64
# ---- fp32 consts layout ----
GE_OFF = 0                       # 1 col: -bias pattern for gate exp (98 rows)
NF = GE_OFF + 1


def _build_consts(gW1, gb1, gW2, gb2, eW1, eb1, eW2, eb2):
    cb = np.zeros((128, NB), dtype=np.float32)
    for u in range(3):
        cb[:, W1_OFF + u * H: W1_OFF + (u + 1) * H] = gW1[u]
    for e in range(4):
        cb[:, W1_OFF + (3 + e) * H: W1_OFF + (4 + e) * H] = eW1[e]
    for k in range(KH):
        for e in range(4):
            cb[:, W2_OFF + (k * 4 + e) * 64: W2_OFF + (k * 4 + e + 1) * 64] = \
                eW2[e, k * 128:(k + 1) * 128, :]
    v = gW2[:, :, 0] - gW2[:, :, 1]          # [3, 512] logit-diff weights
    for k in range(KH):
        sl = slice(k * 128, (k + 1) * 128)
        blk = np.zeros((128, 34), dtype=np.float32)
        blk[:, 0] = v[0, sl]
        blk[:, 1] = v[0, sl]
        blk[:, 32] = -v[0, sl]
        blk[:, 33] = -v[0, sl]
        cb[:, GR_OFF + k * 34: GR_OFF + (k + 1) * 34] = blk
        cb[:, GA_OFF + k * 2] = v[1, sl]
        cb[:, GA_OFF + k * 2 + 1] = -v[1, sl]
        cb[:, GB_OFF2 + k * 2] = v[2, sl]
        cb[:, GB_OFF2 + k * 2 + 1] = -v[2, sl]
    cb[:64, OS_OFF + 0] = 1.0
    cb[64:, OS_OFF + 1] = 1.0
    # broadcast lhsT [2,128]: row0 -> out partitions 0-63, row1 -> 64-127.
    # Replicated at rows 32,33 (matmul needs lhsT/rhs base partitions equal).
    for r0 in (0, 32):
        cb[r0, BC_OFF: BC_OFF + 64] = 1.0
        cb[r0 + 1, BC_OFF + 64: BC_OFF + 128] = 1.0
    p = np.arange(128)
    cb[:, ID_OFF: ID_OFF + 64] = (p[:, None] % 64 == np.arange(64)[None, :])

    # gate exp bias pattern (gb2 diffs; zeros per spec but kept for exactness)
    cf = np.zeros((128, NF), dtype=np.float32)
    db = gb2[:, 0] - gb2[:, 1]               # [3]
    cf[0:2, GE_OFF] = -db[0]
    cf[32:34, GE_OFF] = db[0]
    cf[64, GE_OFF] = -db[1]
    cf[65, GE_OFF] = db[1]
    cf[96, GE_OFF] = -db[2]
    cf[97, GE_OFF] = db[2]
    return cb.astype(ml_dtypes.bfloat16), cf


def _build_nc(n_tiles):
    nc = bacc.Bacc("TRN2", target_bir_lowering=False)
    xt = nc.dram_tensor("xt", [D, BC], BF16, kind="ExternalInput")
    cbd = nc.dram_tensor("cb", [128, NB], BF16, kind="ExternalInput")
    cfd = nc.dram_tensor("cf", [128, NF], F32, kind="ExternalInput")
    outT = nc.dram_tensor("outT", [C, BC], F32, kind="ExternalOutput")

    AF = mybir.ActivationFunctionType
    OP = mybir.AluOpType

    with tile.TileContext(nc) as tc:
        with (
            tc.tile_pool(name="singles", bufs=1) as singles,
            tc.tile_pool(name="xp", bufs=3) as xp,
            tc.tile_pool(name="hp", bufs=3) as hp,
            tc.tile_pool(name="ep", bufs=2) as ep,
            tc.tile_pool(name="sp", bufs=3) as sp,
            tc.tile_pool(name="op", bufs=2) as op_pool,
            tc.tile_pool(name="psL1", bufs=2, space="PSUM") as psL1p,
            tc.tile_pool(name="psE", bufs=2, space="PSUM") as psEp,
            tc.tile_pool(name="psG", bufs=1, space="PSUM") as psGp,
        ):
            cs = singles.tile([128, NB], BF16)
            nc.sync.dma_start(out=cs, in_=cbd[:, :])
            cf = singles.tile([128, NF], F32)
            nc.sync.dma_start(out=cf, in_=cfd[:, :])

            def w1_ap(u, hb):
                a = W1_OFF + u * H + hb * 128
                return cs[:, a: a + 128]

            def w2_ap(k, e):
                a = W2_OFF + (k * 4 + e) * 64
                return cs[:, a: a + 64]

            ones2 = cs[:, OS_OFF: OS_OFF + 2]
            id2 = cs[:, ID_OFF: ID_OFF + 64]

            # cross-iteration state (software pipelining, 2-deep skew)
            E_prev = None          # E(t-1)
            expc_prev = None       # expc(t-1)
            Cb_cur = None          # Cb(t-1), produced in this block
            prod_p1 = None         # at step 4 of block t: prods(t-2)

            def sums_mm(psGb, expc_s):
                # softmax sums on concurrent col tiles (0,0)/(0,32)
                nc.tensor.matmul(psGb[0:2, :], ones2, expc_s[:, 0:TB],
                                 start=True, stop=True, tile_position=(0, 0))
                nc.tensor.matmul(psGb[32:34, :], ones2, expc_s[:, TB:2 * TB],
                                 start=True, stop=True, tile_position=(0, 32))

            def coeff_chain(E_s, psGb):
                t34 = sp.tile([34, TB], F32, tag="t34")
                nc.vector.scalar_tensor_tensor(t34, E_s[64:98, :], 1.0,
                                               psGb[0:34, :],
                                               op0=OP.add, op1=OP.mult)
                m34 = sp.tile([34, TB], F32, tag="m34")
                nc.vector.scalar_tensor_tensor(m34, E_s[0:34, :], 1.0, t34,
                                               op0=OP.add, op1=OP.mult)
                Cf_t = sp.tile([34, TB], F32, tag="C")
                nc.vector.reciprocal_approx_fast(Cf_t, m34)
                Cb = sp.tile([34, TB], BF16, tag="Cb")
                nc.vector.tensor_scalar(Cb, Cf_t, 0.0, None, op0=OP.add)
                return Cb

            def final_mm(psGb, prods, t_out):
                psO = psGb[64:128, :]
                nc.tensor.matmul(psO, id2, prods[0], start=True, stop=False,
                                 tile_position=(0, 64))
                nc.tensor.matmul(psO, id2, prods[1], start=False, stop=True,
                                 tile_position=(0, 64))
                osb = op_pool.tile([64, TB], F32, tag="osb")
                nc.scalar.copy(osb, psO)
                nc.sync.dma_start(out=outT[:, t_out * TB:(t_out + 1) * TB],
                                  in_=osb)

            def bcast_prod(Cb, expc_s):
                prods = []
                for pair in range(2):
                    psBC = psEp.tile([128, TB], F32, tag="e2")
                    bl = cs[32 * pair: 32 * pair + 2, BC_OFF: BC_OFF + 128]
                    nc.tensor.matmul(psBC, bl,
                                     Cb[32 * pair: 32 * pair + 2, :],
                                     start=True, stop=True)
                    prod = sp.tile([128, TB], BF16, tag=f"prod{pair}")
                    nc.vector.tensor_tensor(
                        prod, expc_s[:, pair * TB:(pair + 1) * TB], psBC,
                        op=OP.mult)
                    prods.append(prod)
                return prods

            for t in range(n_tiles):
                # ---- step 1: L1(t), (128,128) mode ----
                xtile = xp.tile([D, TB], BF16, tag="x")
                nc.sync.dma_start(out=xtile, in_=xt[:, t * TB:(t + 1) * TB])
                hsb = {}
                for j in range(14):
                    u, hb0 = (2 * j) // KH, (2 * j) % KH
                    psD = psL1p.tile([128, 2 * TB], F32, tag="l1")
                    nc.tensor.matmul(psD[:, 0:TB], w1_ap(u, hb0), xtile,
                                     start=True, stop=True)
                    nc.tensor.matmul(psD[:, TB:2 * TB], w1_ap(u, hb0 + 1),
                                     xtile, start=True, stop=True)
                    hd = hp.tile([128, 2 * TB], BF16, tag=f"h{j}", bufs=3)
                    if j in (2, 4, 6, 8, 10, 12):
                        nc.vector.tensor_scalar(hd, psD, 0.0, None, op0=OP.max)
                    else:
                        nc.scalar.activation(hd, psD, AF.Relu)
                    hsb[u, hb0] = hd[:, 0:TB]
                    hsb[u, hb0 + 1] = hd[:, TB:2 * TB]

                # ---- step 2: (128,32)-mode group ----
                # gates(t): 4 concurrent col tiles, one psum bank
                psGa = psGp.tile([128, TB], F32, tag="ga")
                for k in range(KH):
                    st, sp_ = (k == 0), (k == KH - 1)
                    ga = GR_OFF + k * 34
                    nc.tensor.matmul(psGa[0:2, :], cs[:, ga: ga + 2],
                                     hsb[0, k], start=st, stop=sp_,
                                     tile_position=(0, 0))
                    nc.tensor.matmul(psGa[32:34, :], cs[:, ga + 32: ga + 34],
                                     hsb[0, k], start=st, stop=sp_,
                                     tile_position=(0, 32))
                    nc.tensor.matmul(psGa[64:66, :],
                                     cs[:, GA_OFF + k * 2: GA_OFF + (k + 1) * 2],
                                     hsb[1, k], start=st, stop=sp_,
                                     tile_position=(0, 64))
                    nc.tensor.matmul(psGa[96:98, :],
                                     cs[:, GB_OFF2 + k * 2: GB_OFF2 + (k + 1) * 2],
                                     hsb[2, k], start=st, stop=sp_,
                                     tile_position=(0, 96))
                psGb = psGp.tile([128, TB], F32, tag="gb")
                if t >= 1:
                    sums_mm(psGb, expc_prev)

                # ---- step 3: gate exp (Scalar) + coeff chain t-1 (Vector) ----
                E = sp.tile([98, TB], F32, tag="E")
                nc.scalar.activation(E, psGa[0:98, :], AF.Exp, scale=-1.0,
                                     bias=cf[0:98, GE_OFF: GE_OFF + 1])
                if t >= 1:
                    Cb_cur = coeff_chain(E_prev, psGb)

                # ---- step 4: (128,64)-mode group: experts(t) + final(t-2) ----
                expc = ep.tile([128, 2 * TB], BF16, tag="exp")
                psEs = []
                for pair in range(2):
                    psE = psEp.tile([128, TB], F32, tag="e2")
                    ua, ub = 3 + 2 * pair, 4 + 2 * pair
                    for k in range(KH):
                        nc.tensor.matmul(psE[0:64, :], w2_ap(k, 2 * pair),
                                         hsb[ua, k], start=(k == 0),
                                         stop=(k == KH - 1),
                                         tile_position=(0, 0))
                        nc.tensor.matmul(psE[64:128, :], w2_ap(k, 2 * pair + 1),
                                         hsb[ub, k], start=(k == 0),
                                         stop=(k == KH - 1),
                                         tile_position=(0, 64))
                    psEs.append(psE)
                if t >= 2:
                    final_mm(psGb, prod_p1, t - 2)

                # ---- step 5: expert exp evacs (eb2 zero per spec) ----
                for pair in range(2):
                    nc.scalar.activation(expc[:, pair * TB:(pair + 1) * TB],
                                         psEs[pair], AF.Exp)

                # ---- step 6: (32,128)-row-mode: bcast(t-1) + prod(t-1) ----
                if t >= 1:
                    prod_p1 = bcast_prod(Cb_cur, expc_prev)

                E_prev = E
                expc_prev = expc

            # ---- epilogue: drain the 2-deep pipeline ----
            # block E1: sums(T-1), coeffs(T-1), final(T-2), bcast(T-1)
            psGb = psGp.tile([128, TB], F32, tag="gb")
            sums_mm(psGb, expc_prev)
            Cb_cur = coeff_chain(E_prev, psGb)
            if n_tiles >= 2:
                final_mm(psGb, prod_p1, n_tiles - 2)
            prod_last = bcast_prod(Cb_cur, expc_prev)
            # block E2: final(T-1)
            psGb2 = psGp.tile([128, TB], F32, tag="gb")
            final_mm(psGb2, prod_last, n_tiles - 1)

    nc.compile()
    return nc


def kernel(x, gW1, gb1, gW2, gb2, eW1, eb1, eW2, eb2, _trace=False):
    x = np.asarray(x, dtype=np.float32)
    cb, cf = _build_consts(
        np.asarray(gW1, np.float32), np.asarray(gb1, np.float32),
        np.asarray(gW2, np.float32), np.asarray(gb2, np.float32),
        np.asarray(eW1, np.float32), np.asarray(eb1, np.float32),
        np.asarray(eW2, np.float32), np.asarray(eb2, np.float32))
    n_rows = x.shape[0]
    bc = n_rows // NCORES
    n_tiles = bc // TB
    assert bc * NCORES == n_rows and n_tiles * TB == bc

    global BC
    BC = bc
    nc = _build_nc(n_tiles)

    xs = x.reshape(NCORES, bc, D)
    in_maps = [
        {"xt": np.ascontiguousarray(xs[c].T).astype(ml_dtypes.bfloat16),
         "cb": cb, "cf": cf}
        for c in range(NCORES)
    ]
    res = run_bass_kernel_spmd(nc, in_maps, core_ids=list(range(NCORES)),
                               trace=_trace)
    out = np.concatenate([r["outT"].T for r in res.results], axis=0)
    kernel.last_results = res
    return np.ascontiguousarray(out.astype(np.float32))
